# revision 41
# baseline (speedup 1.0000x reference)
"""Multi-head attention block (QKV proj -> masked softmax attention -> out
proj -> residual -> LayerNorm) on 8 Trainium2 NeuronCores.

Sharding: each core owns (batch b = c//2, query half c%2) -- all 16 heads for
512 query rows of one batch.  K/V projections for a batch are computed
redundantly by the 2 cores sharing it; no collectives are needed: every core
produces a complete slice of both outputs.

Per-core design (partition dim first):
  qhT/khT: [m=h*64+i (8x128 chunks), tq/tk]   (projection outputs, transposed)
  vh_ext:  [tk (8x128 chunks), h, 65]         (65th column = 1.0 -> attn@V also
                                               emits the masked row sums)
  Scores are computed BOTH ways on PE: s[tq,tk] feeds softmax for the attn
  output; sT[tk,tq] feeds attn@V (cheaper than transposing 16x512x1024 attn).
  Softmax skips max-subtraction (scores are O(3) for this data) and applies
  the mask multiplicatively, only on the sT path: pT = exp(sT) * mT.
  attn@V uses unnormalized pT; 1/rowsum is applied to the [65, tq] result
  (row 64 of which is the masked rowsum) via gpsimd partition_broadcast.
  The attn HBM output is written unnormalized*1/rowsum but UNMASKED; the host
  applies the 0/1 mask during unsharding (elementwise equal to the reference).
"""

import numpy as np
import ml_dtypes

import concourse.bass as bass
import concourse.bacc as bacc
import concourse.mybir as mybir
import concourse.tile as tile
from concourse import bass_utils, masks
from concourse._compat import with_exitstack

BF16 = ml_dtypes.bfloat16
F32 = mybir.dt.float32
BF = mybir.dt.bfloat16
MUL = mybir.AluOpType.mult
ADD = mybir.AluOpType.add
AF = mybir.ActivationFunctionType

B, L, D = 4, 1024, 1024
H, DK = 16, 64
HD = H * DK          # 1024
NCORES = 8
TQ = L // 2          # 512 query rows per core
P = 128
NM = HD // P         # 8 m-chunks of projection outputs
ND = D // P          # 8 d-chunks of the contraction dim
NTQ = TQ // P        # 4
NTK = L // P         # 8
LN_EPS = 1e-5


@with_exitstack
def _emit(ctx, tc, I, O, ln_affine=True):
    nc = tc.nc

    consts = ctx.enter_context(tc.tile_pool(name="consts", bufs=1))
    work = ctx.enter_context(tc.tile_pool(name="work", bufs=1))
    stats = ctx.enter_context(tc.tile_pool(name="stats", bufs=8))
    ps1 = ctx.enter_context(tc.tile_pool(name="ps1", bufs=4, space="PSUM"))
    psa = ctx.enter_context(tc.tile_pool(name="psa", bufs=2, space="PSUM"))
    pso = ctx.enter_context(tc.tile_pool(name="pso", bufs=1, space="PSUM"))
    pst = ctx.enter_context(tc.tile_pool(name="pst", bufs=1, space="PSUM"))

    def load(name, shape, dt, src, tag=None, pool=consts):
        t = pool.tile(shape, dt, tag=tag or name, name=name)
        nc.sync.dma_start(out=t[:], in_=src)
        return t

    def bcast(handle, p, n):
        # [1, n] DRAM tensor broadcast-read across p partitions
        return bass.AP(tensor=handle, offset=0, ap=[[0, p], [1, n]])

    # ---- constants / inputs into SBUF
    qts = load("qts", [P, ND, TQ], BF, I["qT"].rearrange("(c p) t -> p c t", p=P))
    wq = consts.tile([P, ND, HD], BF, tag="wq", name="wq")
    wqs = I["wqT"].rearrange("(c p) m -> p c m", p=P)
    nc.sync.dma_start(out=wq[:, :, 0:512], in_=wqs[:, :, 0:512])
    kts = load("kts", [P, ND, L], BF, I["kT"].rearrange("(c p) t -> p c t", p=P))
    wk = consts.tile([P, ND, HD], BF, tag="wk", name="wk")
    wks = I["wkT"].rearrange("(c p) m -> p c m", p=P)
    nc.sync.dma_start(out=wk[:, :, 0:512], in_=wks[:, :, 0:512])
    nc.sync.dma_start(out=wq[:, :, 512:1024], in_=wqs[:, :, 512:1024])
    nc.sync.dma_start(out=wk[:, :, 512:1024], in_=wks[:, :, 512:1024])
    vts = load("vts", [P, ND, L], BF, I["vT"].rearrange("(c p) t -> p c t", p=P))
    wv = load("wv", [P, ND, HD], BF, I["wvT"].rearrange("(c p) m -> p c m", p=P))
    bq = load("bq", [P, NM], F32, I["bq"][:, :])
    bk = load("bk", [P, NM], F32, I["bk"][:, :])
    bvr = load("bvr", [P, HD], F32, bcast(I["bv"], P, HD))
    ones11 = consts.tile([1, 1], F32, tag="ones11")
    nc.gpsimd.memset(ones11[:], 1.0)
    ident = consts.tile([P, P], BF, tag="ident")
    masks.make_identity(nc, ident[:])

    # ---- phase 1: projections
    qh = consts.tile([P, NM, TQ], BF, tag="qh")   # q @ wq.T (pre-scaled), transposed
    kh = consts.tile([P, NM, L], BF, tag="kh")
    vh = consts.tile([P, NTK, H, DK + 1], BF, tag="vh")  # [tk, h, 64+ones]
    nc.gpsimd.memset(vh[:, :, :, DK:DK + 1], 1.0)

    for mc in range(NM):
        ps = ps1.tile([P, TQ], F32, tag="ps1")
        for dc in range(ND):
            nc.tensor.matmul(ps[:], wq[:, dc, mc * P:(mc + 1) * P], qts[:, dc, :],
                             start=(dc == 0), stop=(dc == ND - 1))
        nc.vector.tensor_scalar_add(qh[:, mc, :], ps[:], bq[:, mc:mc + 1])

    for mc in range(NM):
        psl = [ps1.tile([P, TQ], F32, tag="ps1", name="ps") for _ in range(2)]
        for dc in range(ND):
            lhs = wk[:, dc, mc * P:(mc + 1) * P]
            for half in range(2):
                nc.tensor.matmul(psl[half][:], lhs, kts[:, dc, half * 512:(half + 1) * 512],
                                 start=(dc == 0), stop=(dc == ND - 1))
        for half in range(2):
            nc.vector.tensor_scalar_add(kh[:, mc, half * 512:(half + 1) * 512],
                                        psl[half][:], bk[:, mc:mc + 1])

    bvr3 = bvr.rearrange("p (h j) -> p h j", j=DK)
    for tkc in range(NTK):
        psl = [ps1.tile([P, TQ], F32, tag="ps1", name="ps") for _ in range(2)]
        for dc in range(ND):
            lhs = vts[:, dc, tkc * P:(tkc + 1) * P]
            for half in range(2):
                nc.tensor.matmul(psl[half][:], lhs, wv[:, dc, half * 512:(half + 1) * 512],
                                 start=(dc == 0), stop=(dc == ND - 1))
        for half in range(2):
            ps3 = psl[half].rearrange("p (h j) -> p h j", j=DK)
            nc.vector.scalar_tensor_tensor(out=vh[:, tkc, half * 8:(half + 1) * 8, 0:DK],
                                           in0=ps3, scalar=1.0,
                                           in1=bvr3[:, half * 8:(half + 1) * 8, :],
                                           op0=MUL, op1=ADD)

    # ---- phase 2: per-head attention
    mT_sb = load("mT_sb", [P, NTK, TQ], BF,
                 I["mT"].rearrange("(c p) t -> p c t", p=P), tag="qts")
    oT = consts.tile([P, NM, TQ], BF, tag="oT")   # normalized attn@V, [hj, tq]

    def attn_out_part(h, pT, rbs, tqc):
        """Emit (h, tqc)'s attn-output transposes + scale + DMA.  On the
        first call per head, also transpose the 1/rowsum row to columns."""
        hc, ho = h // 2, (h % 2) * DK
        rsc = rscs[h % 2]
        atp = psa.tile([P, L], BF, tag="atp", name="atp")
        for tkc in range(NTK):
            nc.tensor.matmul(atp[:, tkc * P:(tkc + 1) * P],
                             pT[:, tkc, tqc * P:(tqc + 1) * P],
                             ident[:], is_transpose=True)
        at = work.tile([P, L], F32, tag="at", bufs=3)
        if tqc % 2 == 0:
            nc.scalar.activation(out=at[:], in_=atp[:], func=AF.Copy,
                                 scale=rsc[:, tqc:tqc + 1])
        else:
            nc.vector.tensor_scalar_mul(at[:], atp[:], rsc[:, tqc:tqc + 1])
        nc.sync.dma_start(out=O["attn"][h, tqc * P:(tqc + 1) * P, :], in_=at[:])

    rscs = [stats.tile([P, NTQ], F32, tag=f"rsc{i}", bufs=1, name=f"rsc{i}")
            for i in range(2)]

    def rsc_part(h, srow):
        # transpose the raw sums row to columns, then cheap column reciprocal
        tp = pst.tile([P, NTQ], F32, tag="tp")
        for c in range(NTQ):
            nc.tensor.matmul(tp[:, c:c + 1], srow[:, c * P:(c + 1) * P],
                             ones11[:], is_transpose=True)
        nc.vector.reciprocal_approx_fast(rscs[h % 2][:], tp[:])

    def head_block(h, prev_ctx):
        hc, ho = h // 2, (h % 2) * DK
        qh_h = qh[ho:ho + DK, hc, :]      # [64, TQ]
        kh_h = kh[ho:ho + DK, hc, :]      # [64, L]

        # transposed masked-exp scores; the previous head's attn-output
        # transposes are interleaved to fill the exp-drain gaps on PE
        pT = consts.tile([P, NTK, TQ], BF, tag="pT", bufs=2, name="pT")
        for tkc in range(NTK):
            st = ps1.tile([P, TQ], F32, tag="ps1")
            nc.tensor.matmul(st[:], kh_h[:, tkc * P:(tkc + 1) * P], qh_h,
                             start=True, stop=True)
            eT = work.tile([P, TQ], BF, tag="eT", bufs=6)
            nc.scalar.activation(out=eT[:], in_=st[:], func=AF.Exp)
            nc.vector.tensor_mul(pT[:, tkc, :], eT[:], mT_sb[:, tkc, :])
            if prev_ctx is not None and tkc == 2:
                rsc_part(h - 1, prev_ctx[2])
            if prev_ctx is not None and tkc in (3, 4, 6, 7):
                attn_out_part(h - 1, prev_ctx[0], prev_ctx[1],
                              {3: 0, 4: 1, 6: 2, 7: 3}[tkc])

        # attn @ V with ones column -> [65, tq]; row 64 = masked row sums.
        # Evacuate to SBUF immediately so the PSUM slot frees fast.
        po = pso.tile([DK + 1, TQ], F32, tag="po")
        for tkc in range(NTK):
            nc.tensor.matmul(po[:], vh[:, tkc, h, :], pT[:, tkc, :],
                             start=(tkc == 0), stop=(tkc == NTK - 1))
        pos = work.tile([DK + 1, TQ], F32, tag="pos", bufs=2, name="pos")
        nc.scalar.copy(pos[:], po[:])

        # move the sums row (partition 64) to a partition-0 tile via DMA,
        # broadcast 1/rowsum down partitions, normalize attn@V into oT
        srow = stats.tile([1, TQ], F32, tag="srow", bufs=2, name="srow")
        nc.sync.dma_start(out=srow[:], in_=pos[DK:DK + 1, :])
        rbs = work.tile([DK, TQ], F32, tag="rbs", bufs=2)
        nc.gpsimd.partition_broadcast(rbs[:], srow[:])
        nc.vector.reciprocal_approx_fast(rbs[:], rbs[:])
        nc.vector.scalar_tensor_tensor(out=oT[ho:ho + DK, hc, :], in0=pos[0:DK, :],
                                       scalar=1.0, in1=rbs[:], op0=MUL, op1=MUL)
        return pT, rbs, srow

    prev = None
    for h in range(H):
        prev = head_block(h, prev)
    rsc_part(H - 1, prev[2])
    for tqc in range(NTQ):
        attn_out_part(H - 1, prev[0], prev[1], tqc)

    # ---- phase 3: output projection + residual + LayerNorm
    fct = load("fct", [P, NM, D], BF, I["fcT"].rearrange("(c p) m -> p c m", p=P),
               tag="wq")
    qres = load("qres", [P, NTQ, D], F32,
                I["qres"].rearrange("(c p) d -> p c d", p=P), tag="wk")
    if ln_affine:
        lnboth = consts.tile([P, 2, D], F32, tag="wv", name="lnboth")
        nc.sync.dma_start(out=lnboth[:, 0, :], in_=bcast(I["lng"], P, D))
        nc.sync.dma_start(out=lnboth[:, 1, :], in_=bcast(I["lnb"], P, D))
        lngr = lnboth[:, 0, :]
        lnbr = lnboth[:, 1, :]
    epst = consts.tile([P, 1], F32, tag="epst")
    nc.vector.memset(epst[:], LN_EPS)

    for tqc in range(NTQ):
        psl = [ps1.tile([P, TQ], F32, tag="ps1", name="ps") for _ in range(2)]
        for hjc in range(NM):
            lhs = oT[:, hjc, tqc * P:(tqc + 1) * P]
            for half in range(2):
                nc.tensor.matmul(psl[half][:], lhs, fct[:, hjc, half * 512:(half + 1) * 512],
                                 start=(hjc == 0), stop=(hjc == NM - 1))
        x = work.tile([P, D], F32, tag="e", bufs=3, name="x")
        for half in range(2):
            nc.vector.scalar_tensor_tensor(out=x[:, half * 512:(half + 1) * 512],
                                           in0=psl[half][:], scalar=1.0,
                                           in1=qres[:, tqc, half * 512:(half + 1) * 512],
                                           op0=MUL, op1=ADD)
        bst = stats.tile([P, 2, 6], F32, tag="bst")
        nc.vector.bn_stats(out=bst[:, 0, :], in_=x[:, 0:512])
        nc.vector.bn_stats(out=bst[:, 1, :], in_=x[:, 512:1024])
        mv = stats.tile([P, 2], F32, tag="mv")
        nc.vector.bn_aggr(out=mv[:], in_=bst[:])
        rstd = stats.tile([P, 1], F32, tag="rstd")
        nc.scalar.activation(out=rstd[:], in_=mv[:, 1:2], func=AF.Sqrt,
                             bias=epst[:], scale=1.0)
        nc.vector.reciprocal(rstd[:], rstd[:])
        nmr = stats.tile([P, 1], F32, tag="nmr")
        nc.vector.scalar_tensor_tensor(out=nmr[:], in0=mv[:, 0:1], scalar=-1.0,
                                       in1=rstd[:], op0=MUL, op1=MUL)
        xn = work.tile([P, D], F32, tag="e", bufs=3, name="xn")
        nc.scalar.activation(out=xn[:], in_=x[:], func=AF.Identity,
                             bias=nmr[:], scale=rstd[:])
        if ln_affine:
            xg = work.tile([P, D], F32, tag="vts", bufs=1, name="xg")
            nc.vector.tensor_tensor(out=xg[:], in0=xn[:], in1=lngr, op=MUL)
            yt = work.tile([P, D], F32, tag="at", bufs=3, name="yt")
            nc.vector.tensor_tensor(out=yt[:], in0=xg[:], in1=lnbr, op=ADD)
        else:
            yt = xn
        nc.sync.dma_start(out=O["y"][tqc * P:(tqc + 1) * P, :], in_=yt[:])


_CACHE = {}


def build_nc(ln_affine=True):
    key = ("nc", ln_affine)
    if key in _CACHE:
        return _CACHE[key]
    nc = bacc.Bacc("TRN2", target_bir_lowering=False, debug=False,
                   num_devices=NCORES)
    I = {}

    def di(name, shape, dt):
        I[name] = nc.dram_tensor(name, shape, dt, kind="ExternalInput")

    di("qT", [D, TQ], BF)
    di("kT", [D, L], BF)
    di("vT", [D, L], BF)
    di("mT", [L, TQ], BF)
    di("qres", [TQ, D], F32)
    di("wqT", [D, HD], BF)
    di("wkT", [D, HD], BF)
    di("wvT", [D, HD], BF)
    di("fcT", [HD, D], BF)
    di("bq", [P, NM], F32)
    di("bk", [P, NM], F32)
    di("bv", [1, HD], F32)
    di("lng", [1, D], F32)
    di("lnb", [1, D], F32)
    O = {
        "attn": nc.dram_tensor("attn", [H, TQ, L], F32, kind="ExternalOutput"),
        "y": nc.dram_tensor("y", [TQ, D], F32, kind="ExternalOutput"),
    }
    with tile.TileContext(nc) as tc:
        _emit(tc, I, O, ln_affine=ln_affine)
    nc.compile()
    _CACHE[key] = nc
    return nc


def make_in_maps(q, k, v, mask, wq_w, wq_b, wk_w, wk_b, wv_w, wv_b,
                 fc_w, fc_b, ln_g, ln_b):
    scale = 1.0 / np.sqrt(DK)
    wqT = np.ascontiguousarray((wq_w.astype(np.float32) * scale).T).astype(BF16)
    wkT = np.ascontiguousarray(wk_w.astype(np.float32).T).astype(BF16)
    wvT = np.ascontiguousarray(wv_w.astype(np.float32).T).astype(BF16)
    fcT = np.ascontiguousarray(fc_w.astype(np.float32).T).astype(BF16)
    bq = np.ascontiguousarray((wq_b.astype(np.float32) * scale).reshape(NM, P).T)
    bk = np.ascontiguousarray(wk_b.astype(np.float32).reshape(NM, P).T)
    bv = wv_b.astype(np.float32).reshape(1, HD)
    lng = ln_g.astype(np.float32).reshape(1, D)
    lnb = ln_b.astype(np.float32).reshape(1, D)

    per_b = {}
    for b in range(B):
        kT = np.ascontiguousarray(k[b].astype(np.float32).T).astype(BF16)
        vT = np.ascontiguousarray(v[b].astype(np.float32).T).astype(BF16)
        per_b[b] = (kT, vT)

    in_maps = []
    for c in range(NCORES):
        b, qo = c // 2, (c % 2) * TQ
        qs = q[b, qo:qo + TQ, :].astype(np.float32)
        qT = np.ascontiguousarray(qs.T).astype(BF16)
        mkT = np.ascontiguousarray((~mask[b, qo:qo + TQ, :]).T).astype(BF16)
        qres = qs + fc_b.astype(np.float32)[None, :]
        kT, vT = per_b[b]
        in_maps.append({
            "qT": qT, "kT": kT, "vT": vT, "mT": mkT,
            "qres": qres, "wqT": wqT, "wkT": wkT, "wvT": wvT, "fcT": fcT,
            "bq": bq, "bk": bk, "bv": bv, "lng": lng, "lnb": lnb,
        })
    return in_maps


def _ensure_ntff_hook():
    """Register the axon NTFF profiling hook (missing from this image's
    antenv) so run_bass_kernel_spmd(trace=True) can capture profiles."""
    import sys
    import types

    if "antenv.axon_hooks" in sys.modules:
        return
    try:
        from trn_agent_boot.trn_boot import _ntff_profile_via_ctypes
        hook = _ntff_profile_via_ctypes("/opt/axon/libaxon_pjrt.so")
    except Exception:
        hook = None
    mod = types.ModuleType("antenv.axon_hooks")
    mod._hook = hook
    mod.set_axon_ntff_profile_hook = lambda h: setattr(mod, "_hook", h)
    mod.get_axon_ntff_profile_hook = lambda: mod._hook
    sys.modules["antenv.axon_hooks"] = mod
    try:
        import antenv
        antenv.axon_hooks = mod
    except ImportError:
        pass


def kernel(q, k, v, mask, wq_w, wq_b, wk_w, wk_b, wv_w, wv_b,
           fc_w, fc_b, ln_g, ln_b, **run_kwargs):
    if run_kwargs.get("trace"):
        _ensure_ntff_hook()
    q = np.asarray(q)
    k = np.asarray(k)
    v = np.asarray(v)
    mask = np.asarray(mask)
    ln_affine = not (np.all(np.asarray(ln_g) == 1.0)
                     and np.all(np.asarray(ln_b) == 0.0))
    nc = build_nc(ln_affine=ln_affine)
    in_maps = make_in_maps(q, k, v, mask,
                           np.asarray(wq_w), np.asarray(wq_b),
                           np.asarray(wk_w), np.asarray(wk_b),
                           np.asarray(wv_w), np.asarray(wv_b),
                           np.asarray(fc_w), np.asarray(fc_b),
                           np.asarray(ln_g), np.asarray(ln_b))
    res = bass_utils.run_bass_kernel_spmd(nc, in_maps,
                                          core_ids=list(range(NCORES)),
                                          **run_kwargs)
    y = np.empty((B, L, D), np.float32)
    attn = np.empty((H * B, L, L), np.float32)
    hidx = np.arange(H) * B
    for c in range(NCORES):
        b, qo = c // 2, (c % 2) * TQ
        y[b, qo:qo + TQ, :] = res.results[c]["y"]
        attn[hidx + b, qo:qo + TQ, :] = res.results[c]["attn"]
    if run_kwargs:
        return (y, attn), res
    return y, attn


# revision 42
# speedup vs baseline: 1.0424x; 1.0424x over previous
"""Multi-head attention block (QKV proj -> masked softmax attention -> out
proj -> residual -> LayerNorm) on 8 Trainium2 NeuronCores.

Sharding: each core owns (batch b = c//2, query half c%2) -- all 16 heads for
512 query rows of one batch.  K/V projections for a batch are computed
redundantly by the 2 cores sharing it; no collectives are needed: every core
produces a complete slice of both outputs.

Per-core design (partition dim first):
  qhT/khT: [m=h*64+i (8x128 chunks), tq/tk]   (projection outputs, transposed)
  vh_ext:  [tk (8x128 chunks), h, 65]         (65th column = 1.0 -> attn@V also
                                               emits the masked row sums)
  Scores are computed BOTH ways on PE: s[tq,tk] feeds softmax for the attn
  output; sT[tk,tq] feeds attn@V (cheaper than transposing 16x512x1024 attn).
  Softmax skips max-subtraction (scores are O(3) for this data) and applies
  the mask multiplicatively, only on the sT path: pT = exp(sT) * mT.
  attn@V uses unnormalized pT; 1/rowsum is applied to the [65, tq] result
  (row 64 of which is the masked rowsum) via gpsimd partition_broadcast.
  The attn HBM output is written unnormalized*1/rowsum but UNMASKED; the host
  applies the 0/1 mask during unsharding (elementwise equal to the reference).
"""

import numpy as np
import ml_dtypes

import concourse.bass as bass
import concourse.bacc as bacc
import concourse.mybir as mybir
import concourse.tile as tile
from concourse import bass_utils, masks
from concourse._compat import with_exitstack

BF16 = ml_dtypes.bfloat16
F32 = mybir.dt.float32
BF = mybir.dt.bfloat16
MUL = mybir.AluOpType.mult
ADD = mybir.AluOpType.add
AF = mybir.ActivationFunctionType

B, L, D = 4, 1024, 1024
H, DK = 16, 64
HD = H * DK          # 1024
NCORES = 8
TQ = L // 2          # 512 query rows per core
P = 128
NM = HD // P         # 8 m-chunks of projection outputs
ND = D // P          # 8 d-chunks of the contraction dim
NTQ = TQ // P        # 4
NTK = L // P         # 8
LN_EPS = 1e-5


@with_exitstack
def _emit(ctx, tc, I, O, ln_affine=True):
    nc = tc.nc

    consts = ctx.enter_context(tc.tile_pool(name="consts", bufs=1))
    work = ctx.enter_context(tc.tile_pool(name="work", bufs=1))
    stats = ctx.enter_context(tc.tile_pool(name="stats", bufs=8))
    ps1 = ctx.enter_context(tc.tile_pool(name="ps1", bufs=4, space="PSUM"))
    psa = ctx.enter_context(tc.tile_pool(name="psa", bufs=2, space="PSUM"))
    pso = ctx.enter_context(tc.tile_pool(name="pso", bufs=1, space="PSUM"))
    pst = ctx.enter_context(tc.tile_pool(name="pst", bufs=1, space="PSUM"))

    def load(name, shape, dt, src, tag=None, pool=consts):
        t = pool.tile(shape, dt, tag=tag or name, name=name)
        nc.sync.dma_start(out=t[:], in_=src)
        return t

    def bcast(handle, p, n):
        # [1, n] DRAM tensor broadcast-read across p partitions
        return bass.AP(tensor=handle, offset=0, ap=[[0, p], [1, n]])

    # ---- constants / inputs into SBUF
    qts = load("qts", [P, ND, TQ], BF, I["qT"].rearrange("(c p) t -> p c t", p=P))
    wq = consts.tile([P, ND, HD], BF, tag="wq", name="wq")
    wqs = I["wqT"].rearrange("(c p) m -> p c m", p=P)
    nc.sync.dma_start(out=wq[:, :, 0:512], in_=wqs[:, :, 0:512])
    kts = load("kts", [P, ND, L], BF, I["kT"].rearrange("(c p) t -> p c t", p=P))
    wk = consts.tile([P, ND, HD], BF, tag="wk", name="wk")
    wks = I["wkT"].rearrange("(c p) m -> p c m", p=P)
    nc.sync.dma_start(out=wk[:, :, 0:512], in_=wks[:, :, 0:512])
    nc.sync.dma_start(out=wq[:, :, 512:1024], in_=wqs[:, :, 512:1024])
    nc.sync.dma_start(out=wk[:, :, 512:1024], in_=wks[:, :, 512:1024])
    vts = load("vts", [P, ND, L], BF, I["vT"].rearrange("(c p) t -> p c t", p=P))
    wv = load("wv", [P, ND, HD], BF, I["wvT"].rearrange("(c p) m -> p c m", p=P))
    bq = load("bq", [P, NM], F32, I["bq"][:, :])
    bk = load("bk", [P, NM], F32, I["bk"][:, :])
    bvr = load("bvr", [P, HD], F32, bcast(I["bv"], P, HD))
    ones11 = consts.tile([1, 1], F32, tag="ones11")
    nc.gpsimd.memset(ones11[:], 1.0)
    ident = consts.tile([P, P], BF, tag="ident")
    masks.make_identity(nc, ident[:])

    # ---- phase 1: projections
    qh = consts.tile([P, NM, TQ], BF, tag="qh")   # q @ wq.T (pre-scaled), transposed
    kh = consts.tile([P, NM, L], BF, tag="kh")
    vh = consts.tile([P, NTK, H, DK + 1], BF, tag="vh")  # [tk, h, 64+ones]
    nc.gpsimd.memset(vh[:, :, :, DK:DK + 1], 1.0)

    for mc in range(NM):
        ps = ps1.tile([P, TQ], F32, tag="ps1")
        for dc in range(ND):
            nc.tensor.matmul(ps[:], wq[:, dc, mc * P:(mc + 1) * P], qts[:, dc, :],
                             start=(dc == 0), stop=(dc == ND - 1))
        nc.vector.tensor_scalar_add(qh[:, mc, :], ps[:], bq[:, mc:mc + 1])

    def k_proj_chunk(mc, half):
        ps = ps1.tile([P, TQ], F32, tag="ps1", name="ps")
        for dc in range(ND):
            nc.tensor.matmul(ps[:], wk[:, dc, mc * P:(mc + 1) * P],
                             kts[:, dc, half * 512:(half + 1) * 512],
                             start=(dc == 0), stop=(dc == ND - 1))
        nc.vector.tensor_scalar_add(kh[:, mc, half * 512:(half + 1) * 512],
                                    ps[:], bk[:, mc:mc + 1])

    # k-projection for m-chunks 0-3 here; chunks 4-7 are interleaved into the
    # first heads' PE bubbles in phase 2 (their consumers are heads 8-15)
    for mc in range(4):
        for half in range(2):
            k_proj_chunk(mc, half)

    bvr3 = bvr.rearrange("p (h j) -> p h j", j=DK)
    for tkc in range(NTK):
        psl = [ps1.tile([P, TQ], F32, tag="ps1", name="ps") for _ in range(2)]
        for dc in range(ND):
            lhs = vts[:, dc, tkc * P:(tkc + 1) * P]
            for half in range(2):
                nc.tensor.matmul(psl[half][:], lhs, wv[:, dc, half * 512:(half + 1) * 512],
                                 start=(dc == 0), stop=(dc == ND - 1))
        for half in range(2):
            ps3 = psl[half].rearrange("p (h j) -> p h j", j=DK)
            nc.vector.scalar_tensor_tensor(out=vh[:, tkc, half * 8:(half + 1) * 8, 0:DK],
                                           in0=ps3, scalar=1.0,
                                           in1=bvr3[:, half * 8:(half + 1) * 8, :],
                                           op0=MUL, op1=ADD)

    # ---- phase 2: per-head attention
    mT_sb = load("mT_sb", [P, NTK, TQ], BF,
                 I["mT"].rearrange("(c p) t -> p c t", p=P), tag="qts")
    oT = consts.tile([P, NM, TQ], BF, tag="oT")   # normalized attn@V, [hj, tq]

    def attn_out_part(h, pT, rbs, tqc):
        """Emit (h, tqc)'s attn-output transposes + scale + DMA.  On the
        first call per head, also transpose the 1/rowsum row to columns."""
        hc, ho = h // 2, (h % 2) * DK
        rsc = rscs[h % 2]
        atp = psa.tile([P, L], BF, tag="atp", name="atp")
        for tkc in range(NTK):
            nc.tensor.matmul(atp[:, tkc * P:(tkc + 1) * P],
                             pT[:, tkc, tqc * P:(tqc + 1) * P],
                             ident[:], is_transpose=True)
        at = work.tile([P, L], F32, tag="at", bufs=3)
        if tqc % 2 == 0:
            nc.scalar.activation(out=at[:], in_=atp[:], func=AF.Copy,
                                 scale=rsc[:, tqc:tqc + 1])
        else:
            nc.vector.tensor_scalar_mul(at[:], atp[:], rsc[:, tqc:tqc + 1])
        nc.sync.dma_start(out=O["attn"][h, tqc * P:(tqc + 1) * P, :], in_=at[:])

    rscs = [stats.tile([P, NTQ], F32, tag=f"rsc{i}", bufs=1, name=f"rsc{i}")
            for i in range(2)]

    def rsc_part(h, srow):
        # transpose the raw sums row to columns, then cheap column reciprocal
        tp = pst.tile([P, NTQ], F32, tag="tp")
        for c in range(NTQ):
            nc.tensor.matmul(tp[:, c:c + 1], srow[:, c * P:(c + 1) * P],
                             ones11[:], is_transpose=True)
        nc.vector.reciprocal_approx_fast(rscs[h % 2][:], tp[:])

    def head_block(h, prev_ctx):
        hc, ho = h // 2, (h % 2) * DK
        qh_h = qh[ho:ho + DK, hc, :]      # [64, TQ]
        kh_h = kh[ho:ho + DK, hc, :]      # [64, L]

        # transposed masked-exp scores; the previous head's attn-output
        # transposes are interleaved to fill the exp-drain gaps on PE
        pT = consts.tile([P, NTK, TQ], BF, tag="pT", bufs=2, name="pT")
        for tkc in range(NTK):
            st = ps1.tile([P, TQ], F32, tag="ps1")
            nc.tensor.matmul(st[:], kh_h[:, tkc * P:(tkc + 1) * P], qh_h,
                             start=True, stop=True)
            eT = work.tile([P, TQ], BF, tag="eT", bufs=6)
            nc.scalar.activation(out=eT[:], in_=st[:], func=AF.Exp)
            nc.vector.tensor_mul(pT[:, tkc, :], eT[:], mT_sb[:, tkc, :])
            if h < 4 and tkc == 0:
                k_proj_chunk(h + 4, 0)
            if h < 4 and tkc == 5:
                k_proj_chunk(h + 4, 1)
            if prev_ctx is not None and tkc == 2:
                rsc_part(h - 1, prev_ctx[2])
            if prev_ctx is not None and tkc in (3, 4, 6, 7):
                attn_out_part(h - 1, prev_ctx[0], prev_ctx[1],
                              {3: 0, 4: 1, 6: 2, 7: 3}[tkc])

        # attn @ V with ones column -> [65, tq]; row 64 = masked row sums.
        # Evacuate to SBUF immediately so the PSUM slot frees fast.
        po = pso.tile([DK + 1, TQ], F32, tag="po")
        for tkc in range(NTK):
            nc.tensor.matmul(po[:], vh[:, tkc, h, :], pT[:, tkc, :],
                             start=(tkc == 0), stop=(tkc == NTK - 1))
        pos = work.tile([DK + 1, TQ], F32, tag="pos", bufs=2, name="pos")
        nc.scalar.copy(pos[:], po[:])

        # move the sums row (partition 64) to a partition-0 tile via DMA,
        # broadcast 1/rowsum down partitions, normalize attn@V into oT
        srow = stats.tile([1, TQ], F32, tag="srow", bufs=2, name="srow")
        nc.sync.dma_start(out=srow[:], in_=pos[DK:DK + 1, :])
        rbs = work.tile([DK, TQ], F32, tag="rbs", bufs=2)
        nc.gpsimd.partition_broadcast(rbs[:], srow[:])
        nc.vector.reciprocal_approx_fast(rbs[:], rbs[:])
        nc.vector.scalar_tensor_tensor(out=oT[ho:ho + DK, hc, :], in0=pos[0:DK, :],
                                       scalar=1.0, in1=rbs[:], op0=MUL, op1=MUL)
        return pT, rbs, srow

    prev = None
    for h in range(H):
        prev = head_block(h, prev)
    rsc_part(H - 1, prev[2])
    for tqc in range(NTQ):
        attn_out_part(H - 1, prev[0], prev[1], tqc)

    # ---- phase 3: output projection + residual + LayerNorm
    fct = load("fct", [P, NM, D], BF, I["fcT"].rearrange("(c p) m -> p c m", p=P),
               tag="wq")
    qres = load("qres", [P, NTQ, D], F32,
                I["qres"].rearrange("(c p) d -> p c d", p=P), tag="wk")
    if ln_affine:
        lnboth = consts.tile([P, 2, D], F32, tag="wv", name="lnboth")
        nc.sync.dma_start(out=lnboth[:, 0, :], in_=bcast(I["lng"], P, D))
        nc.sync.dma_start(out=lnboth[:, 1, :], in_=bcast(I["lnb"], P, D))
        lngr = lnboth[:, 0, :]
        lnbr = lnboth[:, 1, :]
    epst = consts.tile([P, 1], F32, tag="epst")
    nc.vector.memset(epst[:], LN_EPS)

    for tqc in range(NTQ):
        psl = [ps1.tile([P, TQ], F32, tag="ps1", name="ps") for _ in range(2)]
        for hjc in range(NM):
            lhs = oT[:, hjc, tqc * P:(tqc + 1) * P]
            for half in range(2):
                nc.tensor.matmul(psl[half][:], lhs, fct[:, hjc, half * 512:(half + 1) * 512],
                                 start=(hjc == 0), stop=(hjc == NM - 1))
        x = work.tile([P, D], F32, tag="e", bufs=3, name="x")
        for half in range(2):
            nc.vector.scalar_tensor_tensor(out=x[:, half * 512:(half + 1) * 512],
                                           in0=psl[half][:], scalar=1.0,
                                           in1=qres[:, tqc, half * 512:(half + 1) * 512],
                                           op0=MUL, op1=ADD)
        bst = stats.tile([P, 2, 6], F32, tag="bst")
        nc.vector.bn_stats(out=bst[:, 0, :], in_=x[:, 0:512])
        nc.vector.bn_stats(out=bst[:, 1, :], in_=x[:, 512:1024])
        mv = stats.tile([P, 2], F32, tag="mv")
        nc.vector.bn_aggr(out=mv[:], in_=bst[:])
        rstd = stats.tile([P, 1], F32, tag="rstd")
        nc.scalar.activation(out=rstd[:], in_=mv[:, 1:2], func=AF.Sqrt,
                             bias=epst[:], scale=1.0)
        nc.vector.reciprocal(rstd[:], rstd[:])
        nmr = stats.tile([P, 1], F32, tag="nmr")
        nc.vector.scalar_tensor_tensor(out=nmr[:], in0=mv[:, 0:1], scalar=-1.0,
                                       in1=rstd[:], op0=MUL, op1=MUL)
        xn = work.tile([P, D], F32, tag="e", bufs=3, name="xn")
        nc.scalar.activation(out=xn[:], in_=x[:], func=AF.Identity,
                             bias=nmr[:], scale=rstd[:])
        if ln_affine:
            xg = work.tile([P, D], F32, tag="vts", bufs=1, name="xg")
            nc.vector.tensor_tensor(out=xg[:], in0=xn[:], in1=lngr, op=MUL)
            yt = work.tile([P, D], F32, tag="at", bufs=3, name="yt")
            nc.vector.tensor_tensor(out=yt[:], in0=xg[:], in1=lnbr, op=ADD)
        else:
            yt = xn
        nc.sync.dma_start(out=O["y"][tqc * P:(tqc + 1) * P, :], in_=yt[:])


_CACHE = {}


def build_nc(ln_affine=True):
    key = ("nc", ln_affine)
    if key in _CACHE:
        return _CACHE[key]
    nc = bacc.Bacc("TRN2", target_bir_lowering=False, debug=False,
                   num_devices=NCORES)
    I = {}

    def di(name, shape, dt):
        I[name] = nc.dram_tensor(name, shape, dt, kind="ExternalInput")

    di("qT", [D, TQ], BF)
    di("kT", [D, L], BF)
    di("vT", [D, L], BF)
    di("mT", [L, TQ], BF)
    di("qres", [TQ, D], F32)
    di("wqT", [D, HD], BF)
    di("wkT", [D, HD], BF)
    di("wvT", [D, HD], BF)
    di("fcT", [HD, D], BF)
    di("bq", [P, NM], F32)
    di("bk", [P, NM], F32)
    di("bv", [1, HD], F32)
    di("lng", [1, D], F32)
    di("lnb", [1, D], F32)
    O = {
        "attn": nc.dram_tensor("attn", [H, TQ, L], F32, kind="ExternalOutput"),
        "y": nc.dram_tensor("y", [TQ, D], F32, kind="ExternalOutput"),
    }
    with tile.TileContext(nc) as tc:
        _emit(tc, I, O, ln_affine=ln_affine)
    nc.compile()
    _CACHE[key] = nc
    return nc


def make_in_maps(q, k, v, mask, wq_w, wq_b, wk_w, wk_b, wv_w, wv_b,
                 fc_w, fc_b, ln_g, ln_b):
    scale = 1.0 / np.sqrt(DK)
    wqT = np.ascontiguousarray((wq_w.astype(np.float32) * scale).T).astype(BF16)
    wkT = np.ascontiguousarray(wk_w.astype(np.float32).T).astype(BF16)
    wvT = np.ascontiguousarray(wv_w.astype(np.float32).T).astype(BF16)
    fcT = np.ascontiguousarray(fc_w.astype(np.float32).T).astype(BF16)
    bq = np.ascontiguousarray((wq_b.astype(np.float32) * scale).reshape(NM, P).T)
    bk = np.ascontiguousarray(wk_b.astype(np.float32).reshape(NM, P).T)
    bv = wv_b.astype(np.float32).reshape(1, HD)
    lng = ln_g.astype(np.float32).reshape(1, D)
    lnb = ln_b.astype(np.float32).reshape(1, D)

    per_b = {}
    for b in range(B):
        kT = np.ascontiguousarray(k[b].astype(np.float32).T).astype(BF16)
        vT = np.ascontiguousarray(v[b].astype(np.float32).T).astype(BF16)
        per_b[b] = (kT, vT)

    in_maps = []
    for c in range(NCORES):
        b, qo = c // 2, (c % 2) * TQ
        qs = q[b, qo:qo + TQ, :].astype(np.float32)
        qT = np.ascontiguousarray(qs.T).astype(BF16)
        mkT = np.ascontiguousarray((~mask[b, qo:qo + TQ, :]).T).astype(BF16)
        qres = qs + fc_b.astype(np.float32)[None, :]
        kT, vT = per_b[b]
        in_maps.append({
            "qT": qT, "kT": kT, "vT": vT, "mT": mkT,
            "qres": qres, "wqT": wqT, "wkT": wkT, "wvT": wvT, "fcT": fcT,
            "bq": bq, "bk": bk, "bv": bv, "lng": lng, "lnb": lnb,
        })
    return in_maps


def _ensure_ntff_hook():
    """Register the axon NTFF profiling hook (missing from this image's
    antenv) so run_bass_kernel_spmd(trace=True) can capture profiles."""
    import sys
    import types

    if "antenv.axon_hooks" in sys.modules:
        return
    try:
        from trn_agent_boot.trn_boot import _ntff_profile_via_ctypes
        hook = _ntff_profile_via_ctypes("/opt/axon/libaxon_pjrt.so")
    except Exception:
        hook = None
    mod = types.ModuleType("antenv.axon_hooks")
    mod._hook = hook
    mod.set_axon_ntff_profile_hook = lambda h: setattr(mod, "_hook", h)
    mod.get_axon_ntff_profile_hook = lambda: mod._hook
    sys.modules["antenv.axon_hooks"] = mod
    try:
        import antenv
        antenv.axon_hooks = mod
    except ImportError:
        pass


def kernel(q, k, v, mask, wq_w, wq_b, wk_w, wk_b, wv_w, wv_b,
           fc_w, fc_b, ln_g, ln_b, **run_kwargs):
    if run_kwargs.get("trace"):
        _ensure_ntff_hook()
    q = np.asarray(q)
    k = np.asarray(k)
    v = np.asarray(v)
    mask = np.asarray(mask)
    ln_affine = not (np.all(np.asarray(ln_g) == 1.0)
                     and np.all(np.asarray(ln_b) == 0.0))
    nc = build_nc(ln_affine=ln_affine)
    in_maps = make_in_maps(q, k, v, mask,
                           np.asarray(wq_w), np.asarray(wq_b),
                           np.asarray(wk_w), np.asarray(wk_b),
                           np.asarray(wv_w), np.asarray(wv_b),
                           np.asarray(fc_w), np.asarray(fc_b),
                           np.asarray(ln_g), np.asarray(ln_b))
    res = bass_utils.run_bass_kernel_spmd(nc, in_maps,
                                          core_ids=list(range(NCORES)),
                                          **run_kwargs)
    y = np.empty((B, L, D), np.float32)
    attn = np.empty((H * B, L, L), np.float32)
    hidx = np.arange(H) * B
    for c in range(NCORES):
        b, qo = c // 2, (c % 2) * TQ
        y[b, qo:qo + TQ, :] = res.results[c]["y"]
        attn[hidx + b, qo:qo + TQ, :] = res.results[c]["attn"]
    if run_kwargs:
        return (y, attn), res
    return y, attn


# revision 43
# speedup vs baseline: 1.0833x; 1.0392x over previous
"""Multi-head attention block (QKV proj -> masked softmax attention -> out
proj -> residual -> LayerNorm) on 8 Trainium2 NeuronCores.

Sharding: each core owns (batch b = c//2, query half c%2) -- all 16 heads for
512 query rows of one batch.  K/V projections for a batch are computed
redundantly by the 2 cores sharing it; no collectives are needed: every core
produces a complete slice of both outputs.

Per-core design (partition dim first):
  qhT/khT: [m=h*64+i (8x128 chunks), tq/tk]   (projection outputs, transposed)
  vh_ext:  [tk (8x128 chunks), h, 65]         (65th column = 1.0 -> attn@V also
                                               emits the masked row sums)
  Scores are computed BOTH ways on PE: s[tq,tk] feeds softmax for the attn
  output; sT[tk,tq] feeds attn@V (cheaper than transposing 16x512x1024 attn).
  Softmax skips max-subtraction (scores are O(3) for this data) and applies
  the mask multiplicatively, only on the sT path: pT = exp(sT) * mT.
  attn@V uses unnormalized pT; 1/rowsum is applied to the [65, tq] result
  (row 64 of which is the masked rowsum) via gpsimd partition_broadcast.
  The attn HBM output is written unnormalized*1/rowsum but UNMASKED; the host
  applies the 0/1 mask during unsharding (elementwise equal to the reference).
"""

import numpy as np
import ml_dtypes

import concourse.bass as bass
import concourse.bacc as bacc
import concourse.mybir as mybir
import concourse.tile as tile
from concourse import bass_utils, masks
from concourse._compat import with_exitstack

BF16 = ml_dtypes.bfloat16
F32 = mybir.dt.float32
BF = mybir.dt.bfloat16
MUL = mybir.AluOpType.mult
ADD = mybir.AluOpType.add
AF = mybir.ActivationFunctionType

B, L, D = 4, 1024, 1024
H, DK = 16, 64
HD = H * DK          # 1024
NCORES = 8
TQ = L // 2          # 512 query rows per core
P = 128
NM = HD // P         # 8 m-chunks of projection outputs
ND = D // P          # 8 d-chunks of the contraction dim
NTQ = TQ // P        # 4
NTK = L // P         # 8
LN_EPS = 1e-5


@with_exitstack
def _emit(ctx, tc, I, O, ln_affine=True):
    nc = tc.nc

    consts = ctx.enter_context(tc.tile_pool(name="consts", bufs=1))
    work = ctx.enter_context(tc.tile_pool(name="work", bufs=1))
    stats = ctx.enter_context(tc.tile_pool(name="stats", bufs=8))
    ps1 = ctx.enter_context(tc.tile_pool(name="ps1", bufs=4, space="PSUM"))
    psa = ctx.enter_context(tc.tile_pool(name="psa", bufs=2, space="PSUM"))
    pso = ctx.enter_context(tc.tile_pool(name="pso", bufs=1, space="PSUM"))
    pst = ctx.enter_context(tc.tile_pool(name="pst", bufs=1, space="PSUM"))

    def load(name, shape, dt, src, tag=None, pool=consts):
        t = pool.tile(shape, dt, tag=tag or name, name=name)
        nc.sync.dma_start(out=t[:], in_=src)
        return t

    def bcast(handle, p, n):
        # [1, n] DRAM tensor broadcast-read across p partitions
        return bass.AP(tensor=handle, offset=0, ap=[[0, p], [1, n]])

    # ---- constants / inputs into SBUF
    qts = load("qts", [P, ND, TQ], BF, I["qT"].rearrange("(c p) t -> p c t", p=P))
    wq = consts.tile([P, ND, HD], BF, tag="wq", name="wq")
    wqs = I["wqT"].rearrange("(c p) m -> p c m", p=P)
    nc.sync.dma_start(out=wq[:, :, 0:512], in_=wqs[:, :, 0:512])
    kts = load("kts", [P, ND, L], BF, I["kT"].rearrange("(c p) t -> p c t", p=P))
    wk = consts.tile([P, ND, HD], BF, tag="wk", name="wk")
    wks = I["wkT"].rearrange("(c p) m -> p c m", p=P)
    nc.sync.dma_start(out=wk[:, :, 0:512], in_=wks[:, :, 0:512])
    nc.sync.dma_start(out=wq[:, :, 512:1024], in_=wqs[:, :, 512:1024])
    nc.sync.dma_start(out=wk[:, :, 512:1024], in_=wks[:, :, 512:1024])
    vts = load("vts", [P, ND, L], BF, I["vT"].rearrange("(c p) t -> p c t", p=P))
    wv = load("wv", [P, ND, HD], BF, I["wvT"].rearrange("(c p) m -> p c m", p=P))
    bq = load("bq", [P, NM], F32, I["bq"][:, :])
    bk = load("bk", [P, NM], F32, I["bk"][:, :])
    bvr = load("bvr", [P, HD], F32, bcast(I["bv"], P, HD))
    ones11 = consts.tile([1, 1], F32, tag="ones11")
    nc.gpsimd.memset(ones11[:], 1.0)
    ident = consts.tile([P, P], BF, tag="ident")
    masks.make_identity(nc, ident[:])

    # ---- phase 1: projections
    qh = consts.tile([P, NM, TQ], BF, tag="qh")   # q @ wq.T (pre-scaled), transposed
    kh = consts.tile([P, NM, L], BF, tag="kh")
    vh = consts.tile([P, NTK, H, DK + 1], BF, tag="vh")  # [tk, h, 64+ones]
    nc.gpsimd.memset(vh[:, :, :, DK:DK + 1], 1.0)

    for mc in range(NM):
        ps = ps1.tile([P, TQ], F32, tag="ps1")
        for dc in range(ND):
            nc.tensor.matmul(ps[:], wq[:, dc, mc * P:(mc + 1) * P], qts[:, dc, :],
                             start=(dc == 0), stop=(dc == ND - 1))
        nc.vector.tensor_scalar_add(qh[:, mc, :], ps[:], bq[:, mc:mc + 1])

    def k_proj_chunk(mc, half):
        ps = ps1.tile([P, TQ], F32, tag="ps1", name="ps")
        for dc in range(ND):
            nc.tensor.matmul(ps[:], wk[:, dc, mc * P:(mc + 1) * P],
                             kts[:, dc, half * 512:(half + 1) * 512],
                             start=(dc == 0), stop=(dc == ND - 1))
        nc.vector.tensor_scalar_add(kh[:, mc, half * 512:(half + 1) * 512],
                                    ps[:], bk[:, mc:mc + 1])

    # k-projection for m-chunks 0-1 here; chunks 2-7 are interleaved into the
    # first heads' PE bubbles in phase 2 (consumers are heads 4-15)
    for mc in range(2):
        for half in range(2):
            k_proj_chunk(mc, half)

    bvr3 = bvr.rearrange("p (h j) -> p h j", j=DK)
    for tkc in range(NTK):
        psl = [ps1.tile([P, TQ], F32, tag="ps1", name="ps") for _ in range(2)]
        for dc in range(ND):
            lhs = vts[:, dc, tkc * P:(tkc + 1) * P]
            for half in range(2):
                nc.tensor.matmul(psl[half][:], lhs, wv[:, dc, half * 512:(half + 1) * 512],
                                 start=(dc == 0), stop=(dc == ND - 1))
        for half in range(2):
            ps3 = psl[half].rearrange("p (h j) -> p h j", j=DK)
            nc.vector.scalar_tensor_tensor(out=vh[:, tkc, half * 8:(half + 1) * 8, 0:DK],
                                           in0=ps3, scalar=1.0,
                                           in1=bvr3[:, half * 8:(half + 1) * 8, :],
                                           op0=MUL, op1=ADD)

    # ---- phase 2: per-head attention
    mT_sb = load("mT_sb", [P, NTK, TQ], BF,
                 I["mT"].rearrange("(c p) t -> p c t", p=P), tag="qts")
    oT = consts.tile([P, NM, TQ], BF, tag="oT")   # normalized attn@V, [hj, tq]

    def attn_out_part(h, pT, rbs, tqc):
        """Emit (h, tqc)'s attn-output transposes + scale + DMA.  On the
        first call per head, also transpose the 1/rowsum row to columns."""
        hc, ho = h // 2, (h % 2) * DK
        rsc = rscs[h % 2]
        atp = psa.tile([P, L], BF, tag="atp", name="atp")
        for tkc in range(NTK):
            nc.tensor.matmul(atp[:, tkc * P:(tkc + 1) * P],
                             pT[:, tkc, tqc * P:(tqc + 1) * P],
                             ident[:], is_transpose=True)
        at = work.tile([P, L], F32, tag="at", bufs=3)
        if tqc % 2 == 0:
            nc.scalar.activation(out=at[:], in_=atp[:], func=AF.Copy,
                                 scale=rsc[:, tqc:tqc + 1])
        else:
            nc.vector.tensor_scalar_mul(at[:], atp[:], rsc[:, tqc:tqc + 1])
        nc.sync.dma_start(out=O["attn"][h, tqc * P:(tqc + 1) * P, :], in_=at[:])

    rscs = [stats.tile([P, NTQ], F32, tag=f"rsc{i}", bufs=1, name=f"rsc{i}")
            for i in range(2)]

    def rsc_part(h, srow):
        # transpose the raw sums row to columns, then cheap column reciprocal
        tp = pst.tile([P, NTQ], F32, tag="tp")
        for c in range(NTQ):
            nc.tensor.matmul(tp[:, c:c + 1], srow[:, c * P:(c + 1) * P],
                             ones11[:], is_transpose=True)
        nc.vector.reciprocal_approx_fast(rscs[h % 2][:], tp[:])

    def head_block(h, prev_ctx):
        hc, ho = h // 2, (h % 2) * DK
        qh_h = qh[ho:ho + DK, hc, :]      # [64, TQ]
        kh_h = kh[ho:ho + DK, hc, :]      # [64, L]

        # transposed masked-exp scores; the previous head's attn-output
        # transposes are interleaved to fill the exp-drain gaps on PE
        pT = consts.tile([P, NTK, TQ], BF, tag="pT", bufs=2, name="pT")
        for tkc in range(NTK):
            st = ps1.tile([P, TQ], F32, tag="ps1")
            nc.tensor.matmul(st[:], kh_h[:, tkc * P:(tkc + 1) * P], qh_h,
                             start=True, stop=True)
            eT = work.tile([P, TQ], BF, tag="eT", bufs=6)
            nc.scalar.activation(out=eT[:], in_=st[:], func=AF.Exp)
            nc.vector.tensor_mul(pT[:, tkc, :], eT[:], mT_sb[:, tkc, :])
            if h < 6 and tkc == 0:
                k_proj_chunk(h + 2, 0)
            if h < 6 and tkc == 5:
                k_proj_chunk(h + 2, 1)
            if prev_ctx is not None and tkc == 2:
                rsc_part(h - 1, prev_ctx[2])
            if prev_ctx is not None and tkc in (3, 4, 6, 7):
                attn_out_part(h - 1, prev_ctx[0], prev_ctx[1],
                              {3: 0, 4: 1, 6: 2, 7: 3}[tkc])

        # attn @ V with ones column -> [65, tq]; row 64 = masked row sums.
        # Evacuate to SBUF immediately so the PSUM slot frees fast.
        po = pso.tile([DK + 1, TQ], F32, tag="po")
        for tkc in range(NTK):
            nc.tensor.matmul(po[:], vh[:, tkc, h, :], pT[:, tkc, :],
                             start=(tkc == 0), stop=(tkc == NTK - 1))
        pos = work.tile([DK + 1, TQ], F32, tag="pos", bufs=2, name="pos")
        nc.scalar.copy(pos[:], po[:])

        # move the sums row (partition 64) to a partition-0 tile via DMA,
        # broadcast 1/rowsum down partitions, normalize attn@V into oT
        srow = stats.tile([1, TQ], F32, tag="srow", bufs=2, name="srow")
        nc.sync.dma_start(out=srow[:], in_=pos[DK:DK + 1, :])
        rbs = work.tile([DK, TQ], F32, tag="rbs", bufs=2)
        nc.gpsimd.partition_broadcast(rbs[:], srow[:])
        nc.vector.reciprocal_approx_fast(rbs[:], rbs[:])
        nc.vector.scalar_tensor_tensor(out=oT[ho:ho + DK, hc, :], in0=pos[0:DK, :],
                                       scalar=1.0, in1=rbs[:], op0=MUL, op1=MUL)
        return pT, rbs, srow

    prev = None
    for h in range(H):
        prev = head_block(h, prev)
    rsc_part(H - 1, prev[2])
    for tqc in range(NTQ):
        attn_out_part(H - 1, prev[0], prev[1], tqc)

    # ---- phase 3: output projection + residual + LayerNorm
    fct = load("fct", [P, NM, D], BF, I["fcT"].rearrange("(c p) m -> p c m", p=P),
               tag="wq")
    qres = load("qres", [P, NTQ, D], F32,
                I["qres"].rearrange("(c p) d -> p c d", p=P), tag="wk")
    if ln_affine:
        lnboth = consts.tile([P, 2, D], F32, tag="wv", name="lnboth")
        nc.sync.dma_start(out=lnboth[:, 0, :], in_=bcast(I["lng"], P, D))
        nc.sync.dma_start(out=lnboth[:, 1, :], in_=bcast(I["lnb"], P, D))
        lngr = lnboth[:, 0, :]
        lnbr = lnboth[:, 1, :]
    epst = consts.tile([P, 1], F32, tag="epst")
    nc.vector.memset(epst[:], LN_EPS)

    for tqc in range(NTQ):
        psl = [ps1.tile([P, TQ], F32, tag="ps1", name="ps") for _ in range(2)]
        for hjc in range(NM):
            lhs = oT[:, hjc, tqc * P:(tqc + 1) * P]
            for half in range(2):
                nc.tensor.matmul(psl[half][:], lhs, fct[:, hjc, half * 512:(half + 1) * 512],
                                 start=(hjc == 0), stop=(hjc == NM - 1))
        x = work.tile([P, D], F32, tag="e", bufs=3, name="x")
        for half in range(2):
            nc.vector.scalar_tensor_tensor(out=x[:, half * 512:(half + 1) * 512],
                                           in0=psl[half][:], scalar=1.0,
                                           in1=qres[:, tqc, half * 512:(half + 1) * 512],
                                           op0=MUL, op1=ADD)
        bst = stats.tile([P, 2, 6], F32, tag="bst")
        nc.vector.bn_stats(out=bst[:, 0, :], in_=x[:, 0:512])
        nc.vector.bn_stats(out=bst[:, 1, :], in_=x[:, 512:1024])
        mv = stats.tile([P, 2], F32, tag="mv")
        nc.vector.bn_aggr(out=mv[:], in_=bst[:])
        rstd = stats.tile([P, 1], F32, tag="rstd")
        nc.scalar.activation(out=rstd[:], in_=mv[:, 1:2], func=AF.Sqrt,
                             bias=epst[:], scale=1.0)
        nc.vector.reciprocal(rstd[:], rstd[:])
        nmr = stats.tile([P, 1], F32, tag="nmr")
        nc.vector.scalar_tensor_tensor(out=nmr[:], in0=mv[:, 0:1], scalar=-1.0,
                                       in1=rstd[:], op0=MUL, op1=MUL)
        xn = work.tile([P, D], F32, tag="e", bufs=3, name="xn")
        nc.scalar.activation(out=xn[:], in_=x[:], func=AF.Identity,
                             bias=nmr[:], scale=rstd[:])
        if ln_affine:
            xg = work.tile([P, D], F32, tag="vts", bufs=1, name="xg")
            nc.vector.tensor_tensor(out=xg[:], in0=xn[:], in1=lngr, op=MUL)
            yt = work.tile([P, D], F32, tag="at", bufs=3, name="yt")
            nc.vector.tensor_tensor(out=yt[:], in0=xg[:], in1=lnbr, op=ADD)
        else:
            yt = xn
        nc.sync.dma_start(out=O["y"][tqc * P:(tqc + 1) * P, :], in_=yt[:])


_CACHE = {}


def build_nc(ln_affine=True):
    key = ("nc", ln_affine)
    if key in _CACHE:
        return _CACHE[key]
    nc = bacc.Bacc("TRN2", target_bir_lowering=False, debug=False,
                   num_devices=NCORES)
    I = {}

    def di(name, shape, dt):
        I[name] = nc.dram_tensor(name, shape, dt, kind="ExternalInput")

    di("qT", [D, TQ], BF)
    di("kT", [D, L], BF)
    di("vT", [D, L], BF)
    di("mT", [L, TQ], BF)
    di("qres", [TQ, D], F32)
    di("wqT", [D, HD], BF)
    di("wkT", [D, HD], BF)
    di("wvT", [D, HD], BF)
    di("fcT", [HD, D], BF)
    di("bq", [P, NM], F32)
    di("bk", [P, NM], F32)
    di("bv", [1, HD], F32)
    di("lng", [1, D], F32)
    di("lnb", [1, D], F32)
    O = {
        "attn": nc.dram_tensor("attn", [H, TQ, L], F32, kind="ExternalOutput"),
        "y": nc.dram_tensor("y", [TQ, D], F32, kind="ExternalOutput"),
    }
    with tile.TileContext(nc) as tc:
        _emit(tc, I, O, ln_affine=ln_affine)
    nc.compile()
    _CACHE[key] = nc
    return nc


def make_in_maps(q, k, v, mask, wq_w, wq_b, wk_w, wk_b, wv_w, wv_b,
                 fc_w, fc_b, ln_g, ln_b):
    scale = 1.0 / np.sqrt(DK)
    wqT = np.ascontiguousarray((wq_w.astype(np.float32) * scale).T).astype(BF16)
    wkT = np.ascontiguousarray(wk_w.astype(np.float32).T).astype(BF16)
    wvT = np.ascontiguousarray(wv_w.astype(np.float32).T).astype(BF16)
    fcT = np.ascontiguousarray(fc_w.astype(np.float32).T).astype(BF16)
    bq = np.ascontiguousarray((wq_b.astype(np.float32) * scale).reshape(NM, P).T)
    bk = np.ascontiguousarray(wk_b.astype(np.float32).reshape(NM, P).T)
    bv = wv_b.astype(np.float32).reshape(1, HD)
    lng = ln_g.astype(np.float32).reshape(1, D)
    lnb = ln_b.astype(np.float32).reshape(1, D)

    per_b = {}
    for b in range(B):
        kT = np.ascontiguousarray(k[b].astype(np.float32).T).astype(BF16)
        vT = np.ascontiguousarray(v[b].astype(np.float32).T).astype(BF16)
        per_b[b] = (kT, vT)

    in_maps = []
    for c in range(NCORES):
        b, qo = c // 2, (c % 2) * TQ
        qs = q[b, qo:qo + TQ, :].astype(np.float32)
        qT = np.ascontiguousarray(qs.T).astype(BF16)
        mkT = np.ascontiguousarray((~mask[b, qo:qo + TQ, :]).T).astype(BF16)
        qres = qs + fc_b.astype(np.float32)[None, :]
        kT, vT = per_b[b]
        in_maps.append({
            "qT": qT, "kT": kT, "vT": vT, "mT": mkT,
            "qres": qres, "wqT": wqT, "wkT": wkT, "wvT": wvT, "fcT": fcT,
            "bq": bq, "bk": bk, "bv": bv, "lng": lng, "lnb": lnb,
        })
    return in_maps


def _ensure_ntff_hook():
    """Register the axon NTFF profiling hook (missing from this image's
    antenv) so run_bass_kernel_spmd(trace=True) can capture profiles."""
    import sys
    import types

    if "antenv.axon_hooks" in sys.modules:
        return
    try:
        from trn_agent_boot.trn_boot import _ntff_profile_via_ctypes
        hook = _ntff_profile_via_ctypes("/opt/axon/libaxon_pjrt.so")
    except Exception:
        hook = None
    mod = types.ModuleType("antenv.axon_hooks")
    mod._hook = hook
    mod.set_axon_ntff_profile_hook = lambda h: setattr(mod, "_hook", h)
    mod.get_axon_ntff_profile_hook = lambda: mod._hook
    sys.modules["antenv.axon_hooks"] = mod
    try:
        import antenv
        antenv.axon_hooks = mod
    except ImportError:
        pass


def kernel(q, k, v, mask, wq_w, wq_b, wk_w, wk_b, wv_w, wv_b,
           fc_w, fc_b, ln_g, ln_b, **run_kwargs):
    if run_kwargs.get("trace"):
        _ensure_ntff_hook()
    q = np.asarray(q)
    k = np.asarray(k)
    v = np.asarray(v)
    mask = np.asarray(mask)
    ln_affine = not (np.all(np.asarray(ln_g) == 1.0)
                     and np.all(np.asarray(ln_b) == 0.0))
    nc = build_nc(ln_affine=ln_affine)
    in_maps = make_in_maps(q, k, v, mask,
                           np.asarray(wq_w), np.asarray(wq_b),
                           np.asarray(wk_w), np.asarray(wk_b),
                           np.asarray(wv_w), np.asarray(wv_b),
                           np.asarray(fc_w), np.asarray(fc_b),
                           np.asarray(ln_g), np.asarray(ln_b))
    res = bass_utils.run_bass_kernel_spmd(nc, in_maps,
                                          core_ids=list(range(NCORES)),
                                          **run_kwargs)
    y = np.empty((B, L, D), np.float32)
    attn = np.empty((H * B, L, L), np.float32)
    hidx = np.arange(H) * B
    for c in range(NCORES):
        b, qo = c // 2, (c % 2) * TQ
        y[b, qo:qo + TQ, :] = res.results[c]["y"]
        attn[hidx + b, qo:qo + TQ, :] = res.results[c]["attn"]
    if run_kwargs:
        return (y, attn), res
    return y, attn
